# revision 15
# baseline (speedup 1.0000x reference)
"""EnsembleDeepSDF MoE-routing kernel for 8 Trainium2 NeuronCores.

Strategy: the harness calls kernel(**inputs) with the full inputs; we do all
routing on the host.  type_vec is sorted, so each expert owns a contiguous
segment of points.  We pick a per-core "phase shape" (tile counts per weight
slot, identical on every core so one SPMD program serves all 8 cores), pack
the 9 experts' segments into the 8*len(shape) single-expert slots, gather
each core's points (padding with point 0), and hand each core its own
pre-transposed/pre-scaled weight slots as inputs.  The device program is a
straight-line Tile kernel: per point-tile, 9 matmul layers with softplus
activations; softplus(u) is computed exactly as max(u, ln(1+exp(u))) using
the ACT engine's exp/ln (one table set) plus DVE for the affine and max.

The torch Softplus(beta=100) is softplus(100*z)/100; we keep activations in
the H = softplus(100*z) domain and fold the 1/100 into the next layer's
weights host-side, so no extra scaling ops run on device.
"""

import numpy as np

T, D_IN, H, OUT, N_HID = 9, 67, 512, 1, 7
BETA = 100.0
N_CORES = 8
PT = 512          # points per tile (one PSUM bank of fp32)
P = 128           # partitions
KC = H // P       # 4 contraction chunks
MC = H // P       # 4 output-feature chunks
PAIR = 2          # point-tiles processed per pipeline step

# matmul dtype knob: "f32" (exact, 4 cyc/col), "f32r" (tf32-ish, 1 cyc/col),
# "bf16"
import os as _os

MM_MODE = _os.environ.get("KERNEL_MM_MODE", "f32r")

_nc_cache = {}
_last_results = None


# --------------------------------------------------------------------------
# Host-side planning: pack expert segments into 8 x len(shape) slots.
# --------------------------------------------------------------------------

def _try_pack(shape, counts):
    """Assign experts to single-expert slots. Slot (c, s) holds shape[s]*PT
    points. Returns {expert: [(core, s, amount), ...]} or None."""
    slots = []  # (capacity, core, s)
    for s, t in enumerate(shape):
        for c in range(N_CORES):
            slots.append([t * PT, c, s])
    experts = sorted(
        [e for e in range(T) if counts[e] > 0], key=lambda e: -counts[e]
    )
    asg = {}
    avail = sorted(slots)  # by capacity asc
    for e in experts:
        need = int(counts[e])
        # smallest single slot that fits
        one = next((sl for sl in avail if sl[0] >= need), None)
        if one is not None:
            asg[e] = [(one[1], one[2], need)]
            avail.remove(one)
            continue
        # greedily take largest slots
        take = []
        rem = need
        pool = sorted(avail, key=lambda sl: -sl[0])
        for sl in pool:
            if rem <= 0:
                break
            amt = min(rem, sl[0])
            take.append((sl[1], sl[2], amt))
            rem -= amt
            avail.remove(sl)
        if rem > 0:
            return None
        asg[e] = take
    return asg


def _plan(counts):
    cands = set()
    for t1 in range(1, 17):
        cands.add((t1,))
        for t2 in range(1, t1 + 1):
            cands.add((t1, t2))
            for t3 in range(1, t2 + 1):
                cands.add((t1, t2, t3))
    for shape in sorted(cands, key=lambda s: (sum(s), len(s))):
        asg = _try_pack(shape, counts)
        if asg is not None:
            return shape, asg
    raise RuntimeError("no feasible slot shape")


# --------------------------------------------------------------------------
# Device program
# --------------------------------------------------------------------------

def _build_nc(shape, mm_mode):
    import concourse.bass as bass
    import concourse.tile as tile
    import concourse.mybir as mybir
    from concourse import bacc

    f32 = mybir.dt.float32
    AF = mybir.ActivationFunctionType
    ALU = mybir.AluOpType
    if mm_mode == "bf16":
        wdt = mybir.dt.bfloat16
        mmdt = mybir.dt.bfloat16
    else:
        wdt = f32
        mmdt = mybir.dt.float32r if mm_mode == "f32r" else f32

    S = len(shape)
    TC = sum(shape)
    NP = TC * PT

    nc = bacc.Bacc("TRN2", target_bir_lowering=False)
    xT_in = nc.dram_tensor("xT", [D_IN, NP], wdt, kind="ExternalInput")
    w0t_in = nc.dram_tensor("w0t", [S, D_IN, H], wdt, kind="ExternalInput")
    wht_in = nc.dram_tensor("wht", [S, N_HID, P, KC, H], wdt, kind="ExternalInput")
    wot_in = nc.dram_tensor("wot", [S, P, KC], wdt, kind="ExternalInput")
    b0v_in = nc.dram_tensor("b0v", [S, P, MC], f32, kind="ExternalInput")
    bhv_in = nc.dram_tensor("bhv", [S, P, N_HID, MC], f32, kind="ExternalInput")
    bov_in = nc.dram_tensor("bov", [S, 1], f32, kind="ExternalInput")
    out_d = nc.dram_tensor("out", [1, NP], f32, kind="ExternalOutput")

    def mm(ap):
        return ap.bitcast(mmdt) if mmdt != wdt else ap

    # pairs of point tiles: (tile_start, ntiles, slot)
    steps = []
    t0 = 0
    for s, nt in enumerate(shape):
        for i in range(0, nt, PAIR):
            steps.append((t0 + i, min(PAIR, nt - i), s))
        t0 += nt

    with tile.TileContext(nc) as tc:
        with (
            tc.tile_pool(name="xin", bufs=3) as xin_pool,
            tc.tile_pool(name="wts", bufs=1) as wts_pool,
            tc.tile_pool(name="whp", bufs=min(9, S * N_HID)) as wh_pool,
            tc.tile_pool(name="uh", bufs=3) as uh_pool,
            tc.tile_pool(name="ebuf", bufs=2) as e_pool,
            tc.tile_pool(name="outp", bufs=2) as out_pool,
            tc.tile_pool(name="ps", bufs=3, space="PSUM") as ps_pool,
            tc.tile_pool(name="ps1", bufs=1, space="PSUM") as ps1_pool,
        ):
            # per-step xT tiles (one DMA writer each, so L0 matmuls wait on
            # a single queue semaphore)
            xT_sb = {}
            for (t0, nt, _s) in steps:
                x_t = xin_pool.tile([D_IN, PAIR * PT], wdt,
                                    name=f"x_{t0}", tag="x")
                nc.sync.dma_start(
                    x_t[:, 0:nt * PT], xT_in[:, t0 * PT:(t0 + nt) * PT]
                )
                xT_sb[t0] = x_t

            w0_sb, wo_sb, b0_sb, bh_sb, bo_sb = [], [], [], [], []
            wh_sb = [[None] * N_HID for _ in range(S)]
            for s in range(S):
                w0_t = wts_pool.tile([D_IN, H], wdt, name=f"w0_{s}")
                nc.sync.dma_start(w0_t[:], w0t_in[s])
                w0_sb.append(w0_t)
                b0_t = wts_pool.tile([P, MC], f32, name=f"b0_{s}")
                nc.sync.dma_start(b0_t[:], b0v_in[s])
                b0_sb.append(b0_t)
                bh_t = wts_pool.tile([P, N_HID, MC], f32, name=f"bh_{s}")
                nc.sync.dma_start(bh_t[:], bhv_in[s])
                bh_sb.append(bh_t)
                wo_t = wts_pool.tile([P, KC], wdt, name=f"wo_{s}")
                nc.sync.dma_start(wo_t[:], wot_in[s])
                wo_sb.append(wo_t)
                bo_t = wts_pool.tile([1, 1], f32, name=f"bo_{s}")
                nc.sync.dma_start(bo_t[:], bov_in[s:s + 1, 0:1])
                bo_sb.append(bo_t)
            for s in range(S):
                for l in range(N_HID):
                    wh_t = wh_pool.tile([P, KC, H], wdt, name=f"wh_{s}_{l}", tag="wh")
                    nc.sync.dma_start(wh_t[:], wht_in[s, l])
                    wh_sb[s][l] = wh_t

            for (t0, nt, s) in steps:
                npts = nt * PT
                h_prev = None
                for l in range(N_HID + 1):
                    # --- matmuls: y = h_prev @ W_l' -> psum ------------------
                    psums = []
                    for mc in range(MC):
                        ps_t = ps_pool.tile(
                            [P, PAIR * PT], f32, name=f"ps_{t0}_{l}_{mc}", tag="ps"
                        )
                        psums.append(ps_t)
                        for kc in range(KC if l > 0 else 1):
                            for ti in range(nt):
                                dst = ps_t[:, ti * PT:(ti + 1) * PT]
                                if l == 0:
                                    lhsT = w0_sb[s][:, mc * P:(mc + 1) * P]
                                    rhs = xT_sb[t0][:, ti * PT:(ti + 1) * PT]
                                    nc.tensor.matmul(
                                        dst, mm(lhsT), mm(rhs), start=True, stop=True
                                    )
                                else:
                                    lhsT = wh_sb[s][l - 1][:, kc, mc * P:(mc + 1) * P]
                                    rhs = h_prev[:, kc, ti * PT:(ti + 1) * PT]
                                    nc.tensor.matmul(
                                        dst, mm(lhsT), mm(rhs),
                                        start=(kc == 0), stop=(kc == KC - 1),
                                    )
                    # --- softplus: H = max(u, ln(1+exp(u))), u = 100y+100b --
                    u_t = uh_pool.tile([P, MC, PAIR * PT], f32,
                                       name=f"u_{t0}_{l}", tag="uh")
                    for mc in range(MC):
                        bias = (b0_sb[s][:, mc:mc + 1] if l == 0
                                else bh_sb[s][:, l - 1, mc:mc + 1])
                        nc.vector.tensor_scalar(
                            u_t[:, mc, 0:npts], psums[mc][:, 0:npts],
                            float(BETA), bias, ALU.mult, ALU.add,
                        )
                    # uc = min(u, 85) so exp never overflows; for u>85 the
                    # final max(u, t) picks u exactly, so clamping is lossless
                    uc_t = e_pool.tile([P, MC, PAIR * PT], f32,
                                       name=f"uc_{t0}_{l}", tag="e")
                    nc.vector.tensor_scalar_min(
                        uc_t[:, :, 0:npts], u_t[:, :, 0:npts], 85.0,
                    )
                    e_t = e_pool.tile([P, MC, PAIR * PT], f32,
                                      name=f"e_{t0}_{l}", tag="e")
                    t_t = e_pool.tile([P, MC, PAIR * PT], f32,
                                      name=f"t_{t0}_{l}", tag="e")
                    nc.scalar.activation(
                        e_t[:, :, 0:npts], uc_t[:, :, 0:npts], AF.Exp)
                    nc.scalar.activation(
                        t_t[:, :, 0:npts], e_t[:, :, 0:npts], AF.Ln, bias=1.0)
                    if mm_mode == "bf16":
                        h_t = uh_pool.tile([P, MC, PAIR * PT], wdt,
                                           name=f"h_{t0}_{l}", tag="hb")
                    else:
                        h_t = u_t  # in-place: u becomes H
                    for mc in range(MC):
                        nc.vector.tensor_max(
                            h_t[:, mc, 0:npts], u_t[:, mc, 0:npts],
                            t_t[:, mc, 0:npts],
                        )
                    h_prev = h_t

                # --- final layer: out = h @ Wo' + bo ------------------------
                ps8 = ps1_pool.tile([1, PAIR * PT], f32, name=f"ps8_{t0}", tag="ps8")
                for kc in range(KC):
                    for ti in range(nt):
                        nc.tensor.matmul(
                            ps8[0:1, ti * PT:(ti + 1) * PT],
                            mm(wo_sb[s][:, kc:kc + 1]),
                            mm(h_prev[:, kc, ti * PT:(ti + 1) * PT]),
                            start=(kc == 0), stop=(kc == KC - 1),
                        )
                o_t = out_pool.tile([1, PAIR * PT], f32, name=f"o_{t0}", tag="o")
                nc.vector.tensor_scalar(
                    o_t[0:1, 0:npts], ps8[0:1, 0:npts],
                    bo_sb[s][0:1, 0:1], None, ALU.add,
                )
                nc.sync.dma_start(
                    out_d[0:1, t0 * PT:t0 * PT + npts], o_t[0:1, 0:npts]
                )
    nc.compile()
    return nc


# --------------------------------------------------------------------------
# kernel()
# --------------------------------------------------------------------------

def kernel(x, type_vec, W0, b0, Wh, bh, Wo, bo):
    from concourse.bass_utils import run_bass_kernel_spmd
    import ml_dtypes

    x = np.ascontiguousarray(np.asarray(x, dtype=np.float32))
    tv = np.asarray(type_vec).astype(np.int64)
    W0 = np.asarray(W0, dtype=np.float32)
    b0 = np.asarray(b0, dtype=np.float32)
    Wh = np.asarray(Wh, dtype=np.float32)
    bh = np.asarray(bh, dtype=np.float32)
    Wo = np.asarray(Wo, dtype=np.float32)
    bo = np.asarray(bo, dtype=np.float32)
    N = x.shape[0]

    counts = np.bincount(tv, minlength=T)
    starts = np.concatenate([[0], np.cumsum(counts)])
    shape, asg = _plan(counts)
    S = len(shape)
    TC = sum(shape)
    NP = TC * PT
    phase_off = np.concatenate([[0], np.cumsum(np.asarray(shape))]) * PT

    # per-core slot -> expert, and gathered point indices
    slot_expert = np.zeros((N_CORES, S), dtype=np.int64)
    gidx = np.full((N_CORES, NP), -1, dtype=np.int64)
    for e, takes in asg.items():
        pos = int(starts[e])
        for (c, s, amt) in takes:
            o = int(phase_off[s])
            gidx[c, o:o + amt] = np.arange(pos, pos + amt)
            slot_expert[c, s] = e
            pos += amt

    np_wdt = ml_dtypes.bfloat16 if MM_MODE == "bf16" else np.float32

    # pre-transposed / pre-scaled weight views per expert
    w0t_e = np.ascontiguousarray(W0.transpose(0, 2, 1))            # [T,67,H]
    whs = (Wh / BETA).astype(np.float32)                           # [T,7,H,H]
    wht_e = np.ascontiguousarray(
        whs.transpose(0, 1, 3, 2).reshape(T, N_HID, KC, P, H).transpose(0, 1, 3, 2, 4)
    )                                                              # [T,7,P,KC,H]
    wot_e = np.ascontiguousarray(
        (Wo / BETA).reshape(T, H).reshape(T, KC, P).transpose(0, 2, 1)
    )                                                              # [T,P,KC]
    b0v_e = np.ascontiguousarray((BETA * b0).reshape(T, MC, P).transpose(0, 2, 1))
    bhv_e = np.ascontiguousarray(
        (BETA * bh).reshape(T, N_HID, MC, P).transpose(0, 3, 1, 2)
    )                                                              # [T,P,7,MC]
    bov_e = bo.reshape(T, 1)

    in_maps = []
    for c in range(N_CORES):
        sel = np.where(gidx[c] >= 0, gidx[c], 0)
        xg = x[sel]                                                # [NP, 67]
        ex = slot_expert[c]
        in_maps.append({
            "xT": np.ascontiguousarray(xg.T).astype(np_wdt),
            "w0t": w0t_e[ex].astype(np_wdt),
            "wht": wht_e[ex].astype(np_wdt),
            "wot": wot_e[ex].astype(np_wdt),
            "b0v": b0v_e[ex],
            "bhv": bhv_e[ex],
            "bov": bov_e[ex],
        })

    key = (shape, MM_MODE)
    if key not in _nc_cache:
        _nc_cache[key] = _build_nc(shape, MM_MODE)
    nc = _nc_cache[key]

    res = run_bass_kernel_spmd(nc, in_maps, core_ids=list(range(N_CORES)))
    global _last_results
    _last_results = res

    out = np.zeros((N, OUT), dtype=np.float32)
    for c in range(N_CORES):
        oc = res.results[c]["out"].reshape(-1)
        m = gidx[c] >= 0
        out[gidx[c][m], 0] = oc[m]
    return out


# revision 18
# speedup vs baseline: 1.6542x; 1.6542x over previous
"""EnsembleDeepSDF MoE-routing kernel for 8 Trainium2 NeuronCores.

Strategy: the harness calls kernel(**inputs) with the full inputs; we do all
routing on the host.  type_vec is sorted, so each expert owns a contiguous
segment of points.  We pick a per-core "phase shape" (tile counts per weight
slot, identical on every core so one SPMD program serves all 8 cores), pack
the 9 experts' segments into the 8*len(shape) single-expert slots, gather
each core's points (padding with point 0), and hand each core its own
pre-transposed/pre-scaled weight slots as inputs.  The device program is a
straight-line Tile kernel: per point-tile, 9 matmul layers with softplus
activations; softplus(u) is computed exactly as max(u, ln(1+exp(u))) using
the ACT engine's exp/ln (one table set) plus DVE for the affine and max.

The torch Softplus(beta=100) is softplus(100*z)/100; we keep activations in
the H = softplus(100*z) domain and fold the 1/100 into the next layer's
weights host-side, so no extra scaling ops run on device.
"""

import numpy as np

T, D_IN, H, OUT, N_HID = 9, 67, 512, 1, 7
BETA = 100.0
N_CORES = 8
PT = 512          # points per tile (one PSUM bank of fp32)
P = 128           # partitions
KC = H // P       # 4 contraction chunks
MC = H // P       # 4 output-feature chunks
PAIR = 2          # point-tiles processed per pipeline step

# matmul dtype knob: "f32" (exact, 4 cyc/col), "f32r" (tf32-ish, 1 cyc/col),
# "bf16"
import os as _os

MM_MODE = _os.environ.get("KERNEL_MM_MODE", "f32r")

_nc_cache = {}
_last_results = None


# --------------------------------------------------------------------------
# Host-side planning: pack expert segments into 8 x len(shape) slots.
# --------------------------------------------------------------------------

def _try_pack(shape, counts):
    """Assign experts to single-expert slots. Slot (c, s) holds shape[s]*PT
    points. Returns {expert: [(core, s, amount), ...]} or None."""
    slots = []  # (capacity, core, s)
    for s, t in enumerate(shape):
        for c in range(N_CORES):
            slots.append([t * PT, c, s])
    experts = sorted(
        [e for e in range(T) if counts[e] > 0], key=lambda e: -counts[e]
    )
    asg = {}
    avail = sorted(slots)  # by capacity asc
    for e in experts:
        need = int(counts[e])
        # smallest single slot that fits
        one = next((sl for sl in avail if sl[0] >= need), None)
        if one is not None:
            asg[e] = [(one[1], one[2], need)]
            avail.remove(one)
            continue
        # greedily take largest slots
        take = []
        rem = need
        pool = sorted(avail, key=lambda sl: -sl[0])
        for sl in pool:
            if rem <= 0:
                break
            amt = min(rem, sl[0])
            take.append((sl[1], sl[2], amt))
            rem -= amt
            avail.remove(sl)
        if rem > 0:
            return None
        asg[e] = take
    return asg


def _plan(counts):
    cands = set()
    for t1 in range(1, 17):
        cands.add((t1,))
        for t2 in range(1, t1 + 1):
            cands.add((t1, t2))
            for t3 in range(1, t2 + 1):
                cands.add((t1, t2, t3))
    for shape in sorted(cands, key=lambda s: (sum(s), len(s))):
        asg = _try_pack(shape, counts)
        if asg is not None:
            return shape, asg
    raise RuntimeError("no feasible slot shape")


# --------------------------------------------------------------------------
# Device program
# --------------------------------------------------------------------------

def _build_nc(shape, mm_mode):
    import concourse.bass as bass
    import concourse.tile as tile
    import concourse.mybir as mybir
    from concourse import bacc

    f32 = mybir.dt.float32
    AF = mybir.ActivationFunctionType
    ALU = mybir.AluOpType
    if mm_mode == "bf16":
        wdt = mybir.dt.bfloat16   # weights/x/h (matmul operands)
        udt = f32                 # u stays f32; h is a separate bf16 tile
    elif mm_mode == "f32r":
        wdt = mybir.dt.float32r
        udt = mybir.dt.float32r   # u doubles as h (in-place max)
    else:
        wdt = f32
        udt = f32

    S = len(shape)
    TC = sum(shape)
    NP = TC * PT

    nc = bacc.Bacc("TRN2", target_bir_lowering=False)
    xT_in = nc.dram_tensor("xT", [D_IN, NP], wdt, kind="ExternalInput")
    w0t_in = nc.dram_tensor("w0t", [S, D_IN, H], wdt, kind="ExternalInput")
    wht_in = nc.dram_tensor("wht", [S, N_HID, P, KC, H], wdt, kind="ExternalInput")
    wot_in = nc.dram_tensor("wot", [S, P, KC], wdt, kind="ExternalInput")
    b0v_in = nc.dram_tensor("b0v", [S, P, MC], f32, kind="ExternalInput")
    bhv_in = nc.dram_tensor("bhv", [S, P, N_HID, MC], f32, kind="ExternalInput")
    bov_in = nc.dram_tensor("bov", [S, 1], f32, kind="ExternalInput")
    out_d = nc.dram_tensor("out", [1, NP], f32, kind="ExternalOutput")

    def mm(ap):
        return ap

    # pairs of point tiles: (tile_start, ntiles, slot)
    steps = []
    t0 = 0
    for s, nt in enumerate(shape):
        for i in range(0, nt, PAIR):
            steps.append((t0 + i, min(PAIR, nt - i), s))
        t0 += nt

    with tile.TileContext(nc) as tc:
        with (
            tc.tile_pool(name="xin", bufs=3) as xin_pool,
            tc.tile_pool(name="wts", bufs=1) as wts_pool,
            tc.tile_pool(name="whp", bufs=min(9, S * N_HID)) as wh_pool,
            tc.tile_pool(name="uh", bufs=3) as uh_pool,
            tc.tile_pool(name="ebuf", bufs=2) as e_pool,
            tc.tile_pool(name="outp", bufs=2) as out_pool,
            tc.tile_pool(name="ps", bufs=3, space="PSUM") as ps_pool,
            tc.tile_pool(name="ps1", bufs=1, space="PSUM") as ps1_pool,
        ):
            # per-step xT tiles (one DMA writer each, so L0 matmuls wait on
            # a single queue semaphore)
            xT_sb = {}
            for (t0, nt, _s) in steps:
                x_t = xin_pool.tile([D_IN, PAIR * PT], wdt,
                                    name=f"x_{t0}", tag="x")
                nc.sync.dma_start(
                    x_t[:, 0:nt * PT], xT_in[:, t0 * PT:(t0 + nt) * PT]
                )
                xT_sb[t0] = x_t

            w0_sb, wo_sb, b0_sb, bh_sb, bo_sb = [], [], [], [], []
            wh_sb = [[None] * N_HID for _ in range(S)]
            for s in range(S):
                w0_t = wts_pool.tile([D_IN, H], wdt, name=f"w0_{s}")
                nc.sync.dma_start(w0_t[:], w0t_in[s])
                w0_sb.append(w0_t)
                b0_t = wts_pool.tile([P, MC], f32, name=f"b0_{s}")
                nc.sync.dma_start(b0_t[:], b0v_in[s])
                b0_sb.append(b0_t)
                bh_t = wts_pool.tile([P, N_HID, MC], f32, name=f"bh_{s}")
                nc.sync.dma_start(bh_t[:], bhv_in[s])
                bh_sb.append(bh_t)
                wo_t = wts_pool.tile([P, KC], wdt, name=f"wo_{s}")
                nc.sync.dma_start(wo_t[:], wot_in[s])
                wo_sb.append(wo_t)
                bo_t = wts_pool.tile([1, 1], f32, name=f"bo_{s}")
                nc.sync.dma_start(bo_t[:], bov_in[s:s + 1, 0:1])
                bo_sb.append(bo_t)
            for s in range(S):
                for l in range(N_HID):
                    wh_t = wh_pool.tile([P, KC, H], wdt, name=f"wh_{s}_{l}", tag="wh")
                    nc.sync.dma_start(wh_t[:], wht_in[s, l])
                    wh_sb[s][l] = wh_t

            for (t0, nt, s) in steps:
                npts = nt * PT
                h_prev = None
                for l in range(N_HID + 1):
                    # --- matmuls: y = h_prev @ W_l' -> psum ------------------
                    psums = []
                    for mc in range(MC):
                        ps_t = ps_pool.tile(
                            [P, PAIR * PT], f32, name=f"ps_{t0}_{l}_{mc}", tag="ps"
                        )
                        psums.append(ps_t)
                        for kc in range(KC if l > 0 else 1):
                            for ti in range(nt):
                                dst = ps_t[:, ti * PT:(ti + 1) * PT]
                                if l == 0:
                                    lhsT = w0_sb[s][:, mc * P:(mc + 1) * P]
                                    rhs = xT_sb[t0][:, ti * PT:(ti + 1) * PT]
                                    nc.tensor.matmul(
                                        dst, mm(lhsT), mm(rhs), start=True, stop=True
                                    )
                                else:
                                    lhsT = wh_sb[s][l - 1][:, kc, mc * P:(mc + 1) * P]
                                    rhs = h_prev[:, kc, ti * PT:(ti + 1) * PT]
                                    nc.tensor.matmul(
                                        dst, mm(lhsT), mm(rhs),
                                        start=(kc == 0), stop=(kc == KC - 1),
                                    )
                    # --- softplus: H = max(u, ln(1+exp(u))), u = 100y+100b --
                    u_t = uh_pool.tile([P, MC, PAIR * PT], udt,
                                       name=f"u_{t0}_{l}", tag="uh")
                    for mc in range(MC):
                        bias = (b0_sb[s][:, mc:mc + 1] if l == 0
                                else bh_sb[s][:, l - 1, mc:mc + 1])
                        nc.vector.tensor_scalar(
                            u_t[:, mc, 0:npts], psums[mc][:, 0:npts],
                            float(BETA), bias, ALU.mult, ALU.add,
                        )
                    # uc = min(u, 85) so exp never overflows; for u>85 the
                    # final max(u, t) picks u exactly, so clamping is lossless
                    uc_t = e_pool.tile([P, MC, PAIR * PT], f32,
                                       name=f"uc_{t0}_{l}", tag="e")
                    nc.vector.tensor_scalar_min(
                        uc_t[:, :, 0:npts], u_t[:, :, 0:npts], 85.0,
                    )
                    e_t = e_pool.tile([P, MC, PAIR * PT], f32,
                                      name=f"e_{t0}_{l}", tag="e")
                    t_t = e_pool.tile([P, MC, PAIR * PT], f32,
                                      name=f"t_{t0}_{l}", tag="e")
                    nc.scalar.activation(
                        e_t[:, :, 0:npts], uc_t[:, :, 0:npts], AF.Exp)
                    nc.scalar.activation(
                        t_t[:, :, 0:npts], e_t[:, :, 0:npts], AF.Ln, bias=1.0)
                    if mm_mode == "bf16":
                        h_t = uh_pool.tile([P, MC, PAIR * PT], wdt,
                                           name=f"h_{t0}_{l}", tag="hb")
                    else:
                        h_t = u_t  # in-place: u becomes H
                    for mc in range(MC):
                        nc.vector.tensor_max(
                            h_t[:, mc, 0:npts], u_t[:, mc, 0:npts],
                            t_t[:, mc, 0:npts],
                        )
                    h_prev = h_t

                # --- final layer: out = h @ Wo' + bo ------------------------
                ps8 = ps1_pool.tile([1, PAIR * PT], f32, name=f"ps8_{t0}", tag="ps8")
                for kc in range(KC):
                    for ti in range(nt):
                        nc.tensor.matmul(
                            ps8[0:1, ti * PT:(ti + 1) * PT],
                            mm(wo_sb[s][:, kc:kc + 1]),
                            mm(h_prev[:, kc, ti * PT:(ti + 1) * PT]),
                            start=(kc == 0), stop=(kc == KC - 1),
                        )
                o_t = out_pool.tile([1, PAIR * PT], f32, name=f"o_{t0}", tag="o")
                nc.vector.tensor_scalar(
                    o_t[0:1, 0:npts], ps8[0:1, 0:npts],
                    bo_sb[s][0:1, 0:1], None, ALU.add,
                )
                nc.sync.dma_start(
                    out_d[0:1, t0 * PT:t0 * PT + npts], o_t[0:1, 0:npts]
                )
    nc.compile()
    return nc


# --------------------------------------------------------------------------
# kernel()
# --------------------------------------------------------------------------

def kernel(x, type_vec, W0, b0, Wh, bh, Wo, bo):
    from concourse.bass_utils import run_bass_kernel_spmd
    import ml_dtypes

    x = np.ascontiguousarray(np.asarray(x, dtype=np.float32))
    tv = np.asarray(type_vec).astype(np.int64)
    W0 = np.asarray(W0, dtype=np.float32)
    b0 = np.asarray(b0, dtype=np.float32)
    Wh = np.asarray(Wh, dtype=np.float32)
    bh = np.asarray(bh, dtype=np.float32)
    Wo = np.asarray(Wo, dtype=np.float32)
    bo = np.asarray(bo, dtype=np.float32)
    N = x.shape[0]

    counts = np.bincount(tv, minlength=T)
    starts = np.concatenate([[0], np.cumsum(counts)])
    shape, asg = _plan(counts)
    S = len(shape)
    TC = sum(shape)
    NP = TC * PT
    phase_off = np.concatenate([[0], np.cumsum(np.asarray(shape))]) * PT

    # per-core slot -> expert, and gathered point indices
    slot_expert = np.zeros((N_CORES, S), dtype=np.int64)
    gidx = np.full((N_CORES, NP), -1, dtype=np.int64)
    for e, takes in asg.items():
        pos = int(starts[e])
        for (c, s, amt) in takes:
            o = int(phase_off[s])
            gidx[c, o:o + amt] = np.arange(pos, pos + amt)
            slot_expert[c, s] = e
            pos += amt

    np_wdt = ml_dtypes.bfloat16 if MM_MODE == "bf16" else np.float32

    # pre-transposed / pre-scaled weight views per expert
    w0t_e = np.ascontiguousarray(W0.transpose(0, 2, 1))            # [T,67,H]
    whs = (Wh / BETA).astype(np.float32)                           # [T,7,H,H]
    wht_e = np.ascontiguousarray(
        whs.transpose(0, 1, 3, 2).reshape(T, N_HID, KC, P, H).transpose(0, 1, 3, 2, 4)
    )                                                              # [T,7,P,KC,H]
    wot_e = np.ascontiguousarray(
        (Wo / BETA).reshape(T, H).reshape(T, KC, P).transpose(0, 2, 1)
    )                                                              # [T,P,KC]
    b0v_e = np.ascontiguousarray((BETA * b0).reshape(T, MC, P).transpose(0, 2, 1))
    bhv_e = np.ascontiguousarray(
        (BETA * bh).reshape(T, N_HID, MC, P).transpose(0, 3, 1, 2)
    )                                                              # [T,P,7,MC]
    bov_e = bo.reshape(T, 1)

    in_maps = []
    for c in range(N_CORES):
        sel = np.where(gidx[c] >= 0, gidx[c], 0)
        xg = x[sel]                                                # [NP, 67]
        ex = slot_expert[c]
        in_maps.append({
            "xT": np.ascontiguousarray(xg.T).astype(np_wdt),
            "w0t": w0t_e[ex].astype(np_wdt),
            "wht": wht_e[ex].astype(np_wdt),
            "wot": wot_e[ex].astype(np_wdt),
            "b0v": b0v_e[ex],
            "bhv": bhv_e[ex],
            "bov": bov_e[ex],
        })

    key = (shape, MM_MODE)
    if key not in _nc_cache:
        _nc_cache[key] = _build_nc(shape, MM_MODE)
    nc = _nc_cache[key]

    res = run_bass_kernel_spmd(nc, in_maps, core_ids=list(range(N_CORES)))
    global _last_results
    _last_results = res

    out = np.zeros((N, OUT), dtype=np.float32)
    for c in range(N_CORES):
        oc = res.results[c]["out"].reshape(-1)
        m = gidx[c] >= 0
        out[gidx[c][m], 0] = oc[m]
    return out


# revision 20
# speedup vs baseline: 2.4703x; 1.4934x over previous
"""EnsembleDeepSDF MoE-routing kernel for 8 Trainium2 NeuronCores.

Strategy: the harness calls kernel(**inputs) with the full inputs; we do all
routing on the host.  type_vec is sorted, so each expert owns a contiguous
segment of points.  We pick a per-core "phase shape" (tile counts per weight
slot, identical on every core so one SPMD program serves all 8 cores), pack
the 9 experts' segments into the 8*len(shape) single-expert slots, gather
each core's points (padding with point 0), and hand each core its own
pre-transposed/pre-scaled weight slots as inputs.  The device program is a
straight-line Tile kernel: per point-tile, 9 matmul layers with softplus
activations; softplus(u) is computed exactly as max(u, ln(1+exp(u))) using
the ACT engine's exp/ln (one table set) plus DVE for the affine and max.

The torch Softplus(beta=100) is softplus(100*z)/100; we keep activations in
the H = softplus(100*z) domain and fold the 1/100 into the next layer's
weights host-side, so no extra scaling ops run on device.
"""

import numpy as np

T, D_IN, H, OUT, N_HID = 9, 67, 512, 1, 7
BETA = 100.0
N_CORES = 8
PT = 512          # points per tile (one PSUM bank of fp32)
P = 128           # partitions
KC = H // P       # 4 contraction chunks
MC = H // P       # 4 output-feature chunks
PAIR = 2          # point-tiles processed per pipeline step

# matmul dtype knob: "f32" (exact, 4 cyc/col), "f32r" (tf32-ish, 1 cyc/col),
# "bf16"
import os as _os

MM_MODE = _os.environ.get("KERNEL_MM_MODE", "f32r")

_nc_cache = {}
_last_results = None


# --------------------------------------------------------------------------
# Host-side planning: pack expert segments into 8 x len(shape) slots.
# --------------------------------------------------------------------------

def _try_pack(shape, counts):
    """Assign experts to single-expert slots. Slot (c, s) holds shape[s]*PT
    points. Returns {expert: [(core, s, amount), ...]} or None."""
    slots = []  # (capacity, core, s)
    for s, t in enumerate(shape):
        for c in range(N_CORES):
            slots.append([t * PT, c, s])
    experts = sorted(
        [e for e in range(T) if counts[e] > 0], key=lambda e: -counts[e]
    )
    asg = {}
    avail = sorted(slots)  # by capacity asc
    for e in experts:
        need = int(counts[e])
        # smallest single slot that fits
        one = next((sl for sl in avail if sl[0] >= need), None)
        if one is not None:
            asg[e] = [(one[1], one[2], need)]
            avail.remove(one)
            continue
        # greedily take largest slots
        take = []
        rem = need
        pool = sorted(avail, key=lambda sl: -sl[0])
        for sl in pool:
            if rem <= 0:
                break
            amt = min(rem, sl[0])
            take.append((sl[1], sl[2], amt))
            rem -= amt
            avail.remove(sl)
        if rem > 0:
            return None
        asg[e] = take
    return asg


def _plan(counts):
    cands = set()
    for t1 in range(1, 17):
        cands.add((t1,))
        for t2 in range(1, t1 + 1):
            cands.add((t1, t2))
            for t3 in range(1, t2 + 1):
                cands.add((t1, t2, t3))
    for shape in sorted(cands, key=lambda s: (sum(s), len(s))):
        asg = _try_pack(shape, counts)
        if asg is not None:
            return shape, asg
    raise RuntimeError("no feasible slot shape")


# --------------------------------------------------------------------------
# Device program
# --------------------------------------------------------------------------

def _build_nc(shape, mm_mode):
    import concourse.bass as bass
    import concourse.tile as tile
    import concourse.mybir as mybir
    from concourse import bacc

    f32 = mybir.dt.float32
    AF = mybir.ActivationFunctionType
    ALU = mybir.AluOpType
    if mm_mode == "bf16":
        wdt = mybir.dt.bfloat16   # weights/x/h (matmul operands)
        udt = f32                 # u stays f32; h is a separate bf16 tile
    elif mm_mode == "f32r":
        wdt = mybir.dt.float32r
        udt = mybir.dt.float32r   # u doubles as h (in-place max)
    else:
        wdt = f32
        udt = f32

    S = len(shape)
    TC = sum(shape)
    NP = TC * PT

    nc = bacc.Bacc("TRN2", target_bir_lowering=False)
    xT_in = nc.dram_tensor("xT", [D_IN, NP], wdt, kind="ExternalInput")
    w0t_in = nc.dram_tensor("w0t", [S, D_IN, H], wdt, kind="ExternalInput")
    wht_in = nc.dram_tensor("wht", [S, N_HID, P, KC, H], wdt, kind="ExternalInput")
    wot_in = nc.dram_tensor("wot", [S, P, KC], wdt, kind="ExternalInput")
    b0v_in = nc.dram_tensor("b0v", [S, P, MC], f32, kind="ExternalInput")
    bhv_in = nc.dram_tensor("bhv", [S, P, N_HID, MC], f32, kind="ExternalInput")
    bov_in = nc.dram_tensor("bov", [S, 1], f32, kind="ExternalInput")
    out_d = nc.dram_tensor("out", [1, NP], f32, kind="ExternalOutput")

    def mm(ap):
        return ap

    # pairs of point tiles: (tile_start, ntiles, slot)
    steps = []
    t0 = 0
    for s, nt in enumerate(shape):
        for i in range(0, nt, PAIR):
            steps.append((t0 + i, min(PAIR, nt - i), s))
        t0 += nt

    with tile.TileContext(nc) as tc:
        with (
            tc.tile_pool(name="xin", bufs=2) as xin_pool,
            tc.tile_pool(name="wts", bufs=1) as wts_pool,
            tc.tile_pool(name="whp", bufs=min(8, S * N_HID)) as wh_pool,
            tc.tile_pool(name="uh", bufs=4) as uh_pool,
            tc.tile_pool(name="ebuf", bufs=2) as e_pool,
            tc.tile_pool(name="outp", bufs=2) as out_pool,
            tc.tile_pool(name="ps", bufs=3, space="PSUM") as ps_pool,
            tc.tile_pool(name="ps1", bufs=1, space="PSUM") as ps1_pool,
        ):
            w0_sb, wo_sb, b0_sb, bh_sb, bo_sb = [], [], [], [], []
            wh_sb = [[None] * N_HID for _ in range(S)]
            for s in range(S):
                w0_t = wts_pool.tile([D_IN, H], wdt, name=f"w0_{s}")
                nc.sync.dma_start(w0_t[:], w0t_in[s])
                w0_sb.append(w0_t)
                b0_t = wts_pool.tile([P, MC], f32, name=f"b0_{s}")
                nc.sync.dma_start(b0_t[:], b0v_in[s])
                b0_sb.append(b0_t)
                bh_t = wts_pool.tile([P, N_HID, MC], f32, name=f"bh_{s}")
                nc.sync.dma_start(bh_t[:], bhv_in[s])
                bh_sb.append(bh_t)
                wo_t = wts_pool.tile([P, KC], wdt, name=f"wo_{s}")
                nc.sync.dma_start(wo_t[:], wot_in[s])
                wo_sb.append(wo_t)
                bo_t = wts_pool.tile([1, 1], f32, name=f"bo_{s}")
                nc.sync.dma_start(bo_t[:], bov_in[s:s + 1, 0:1])
                bo_sb.append(bo_t)
            for s in range(S):
                for l in range(N_HID):
                    wh_t = wh_pool.tile([P, KC, H], wdt, name=f"wh_{s}_{l}", tag="wh")
                    nc.sync.dma_start(wh_t[:], wht_in[s, l])
                    wh_sb[s][l] = wh_t

            # Software pipeline: interleave the layers of two steps so PE runs
            # one step's matmuls while the other's softplus chain (DVE->ACT->
            # DVE) is in flight.
            groups = [steps[i:i + 2] for i in range(0, len(steps), 2)]
            xT_sb = {}
            h_cur = {}

            def emit_layer(t0, nt, s, l):
                npts = nt * PT
                h_prev = h_cur.get(t0)
                psums = []
                for mc in range(MC):
                    ps_t = ps_pool.tile(
                        [P, PAIR * PT], f32, name=f"ps_{t0}_{l}_{mc}", tag="ps"
                    )
                    psums.append(ps_t)
                    for kc in range(KC if l > 0 else 1):
                        for ti in range(nt):
                            dst = ps_t[:, ti * PT:(ti + 1) * PT]
                            if l == 0:
                                lhsT = w0_sb[s][:, mc * P:(mc + 1) * P]
                                rhs = xT_sb[t0][:, ti * PT:(ti + 1) * PT]
                                nc.tensor.matmul(
                                    dst, lhsT, rhs, start=True, stop=True
                                )
                            else:
                                lhsT = wh_sb[s][l - 1][:, kc, mc * P:(mc + 1) * P]
                                rhs = h_prev[:, kc, ti * PT:(ti + 1) * PT]
                                nc.tensor.matmul(
                                    dst, lhsT, rhs,
                                    start=(kc == 0), stop=(kc == KC - 1),
                                )
                # --- softplus: H = max(u, ln(1+exp(min(u,85)))), u=100y+100b
                u_t = uh_pool.tile([P, MC, PAIR * PT], udt,
                                   name=f"u_{t0}_{l}", tag="uh")
                for mc in range(MC):
                    bias = (b0_sb[s][:, mc:mc + 1] if l == 0
                            else bh_sb[s][:, l - 1, mc:mc + 1])
                    nc.vector.tensor_scalar(
                        u_t[:, mc, 0:npts], psums[mc][:, 0:npts],
                        float(BETA), bias, ALU.mult, ALU.add,
                    )
                # clamp is lossless: for u>85 the final max picks u exactly
                uc_t = e_pool.tile([P, MC, PAIR * PT], f32,
                                   name=f"uc_{t0}_{l}", tag="e")
                nc.vector.tensor_scalar_min(
                    uc_t[:, :, 0:npts], u_t[:, :, 0:npts], 85.0,
                )
                e_t = e_pool.tile([P, MC, PAIR * PT], f32,
                                  name=f"e_{t0}_{l}", tag="e")
                t_t = e_pool.tile([P, MC, PAIR * PT], f32,
                                  name=f"t_{t0}_{l}", tag="e")
                nc.scalar.activation(
                    e_t[:, :, 0:npts], uc_t[:, :, 0:npts], AF.Exp)
                nc.scalar.activation(
                    t_t[:, :, 0:npts], e_t[:, :, 0:npts], AF.Ln, bias=1.0)
                if mm_mode == "bf16":
                    h_t = uh_pool.tile([P, MC, PAIR * PT], wdt,
                                       name=f"h_{t0}_{l}", tag="hb")
                else:
                    h_t = u_t  # in-place: u becomes H
                for mc in range(MC):
                    nc.vector.tensor_max(
                        h_t[:, mc, 0:npts], u_t[:, mc, 0:npts],
                        t_t[:, mc, 0:npts],
                    )
                h_cur[t0] = h_t

            def emit_final(t0, nt, s):
                npts = nt * PT
                h_prev = h_cur[t0]
                ps8 = ps1_pool.tile([1, PAIR * PT], f32, name=f"ps8_{t0}", tag="ps8")
                for kc in range(KC):
                    for ti in range(nt):
                        nc.tensor.matmul(
                            ps8[0:1, ti * PT:(ti + 1) * PT],
                            wo_sb[s][:, kc:kc + 1],
                            h_prev[:, kc, ti * PT:(ti + 1) * PT],
                            start=(kc == 0), stop=(kc == KC - 1),
                        )
                o_t = out_pool.tile([1, PAIR * PT], f32, name=f"o_{t0}", tag="o")
                nc.vector.tensor_scalar(
                    o_t[0:1, 0:npts], ps8[0:1, 0:npts],
                    bo_sb[s][0:1, 0:1], None, ALU.add,
                )
                nc.sync.dma_start(
                    out_d[0:1, t0 * PT:t0 * PT + npts], o_t[0:1, 0:npts]
                )

            for grp in groups:
                for (t0, nt, _s) in grp:
                    x_t = xin_pool.tile([D_IN, PAIR * PT], wdt,
                                        name=f"x_{t0}", tag="x")
                    nc.sync.dma_start(
                        x_t[:, 0:nt * PT], xT_in[:, t0 * PT:(t0 + nt) * PT]
                    )
                    xT_sb[t0] = x_t
                for l in range(N_HID + 1):
                    for (t0, nt, s) in grp:
                        emit_layer(t0, nt, s, l)
                for (t0, nt, s) in grp:
                    emit_final(t0, nt, s)

    # Pin Exp+Ln to the one table set containing both, so the ACT engine
    # doesn't reload tables between every exp and ln.
    import concourse.bacc as bacc_mod
    import concourse.hw_specs as hw_specs
    _real_tables = hw_specs.get_activation_tables
    _keep = "natural_log_exp_and_others"

    def _pinned_tables(arch):
        t = _real_tables(arch)
        return {
            name: (funcs if name == _keep else (funcs - {AF.Exp, AF.Ln}))
            for name, funcs in t.items()
        }

    bacc_mod.get_activation_tables = _pinned_tables
    try:
        nc.compile()
    finally:
        bacc_mod.get_activation_tables = _real_tables
    return nc


# --------------------------------------------------------------------------
# kernel()
# --------------------------------------------------------------------------

def _maybe_patch_ldw_opt():
    """Optionally flip walrus's --enable-ldw-opt (dedups back-to-back
    LDWEIGHTS of the same stationary operand). Gated by env for A/B."""
    import concourse.bass_utils as bu

    if _os.environ.get("KERNEL_LDW_OPT") != "1":
        return
    if getattr(bu.run_command, "_ldw_patched", False):
        return
    orig = bu.run_command

    def patched(argv, **kw):
        argv = [
            "--enable-ldw-opt=true" if a == "--enable-ldw-opt=false" else a
            for a in argv
        ]
        return orig(argv, **kw)

    patched._ldw_patched = True
    bu.run_command = patched


def kernel(x, type_vec, W0, b0, Wh, bh, Wo, bo):
    from concourse.bass_utils import run_bass_kernel_spmd
    import ml_dtypes

    _maybe_patch_ldw_opt()

    x = np.ascontiguousarray(np.asarray(x, dtype=np.float32))
    tv = np.asarray(type_vec).astype(np.int64)
    W0 = np.asarray(W0, dtype=np.float32)
    b0 = np.asarray(b0, dtype=np.float32)
    Wh = np.asarray(Wh, dtype=np.float32)
    bh = np.asarray(bh, dtype=np.float32)
    Wo = np.asarray(Wo, dtype=np.float32)
    bo = np.asarray(bo, dtype=np.float32)
    N = x.shape[0]

    counts = np.bincount(tv, minlength=T)
    starts = np.concatenate([[0], np.cumsum(counts)])
    shape, asg = _plan(counts)
    S = len(shape)
    TC = sum(shape)
    NP = TC * PT
    phase_off = np.concatenate([[0], np.cumsum(np.asarray(shape))]) * PT

    # per-core slot -> expert, and gathered point indices
    slot_expert = np.zeros((N_CORES, S), dtype=np.int64)
    gidx = np.full((N_CORES, NP), -1, dtype=np.int64)
    for e, takes in asg.items():
        pos = int(starts[e])
        for (c, s, amt) in takes:
            o = int(phase_off[s])
            gidx[c, o:o + amt] = np.arange(pos, pos + amt)
            slot_expert[c, s] = e
            pos += amt

    np_wdt = ml_dtypes.bfloat16 if MM_MODE == "bf16" else np.float32

    # pre-transposed / pre-scaled weight views per expert
    w0t_e = np.ascontiguousarray(W0.transpose(0, 2, 1))            # [T,67,H]
    whs = (Wh / BETA).astype(np.float32)                           # [T,7,H,H]
    wht_e = np.ascontiguousarray(
        whs.transpose(0, 1, 3, 2).reshape(T, N_HID, KC, P, H).transpose(0, 1, 3, 2, 4)
    )                                                              # [T,7,P,KC,H]
    wot_e = np.ascontiguousarray(
        (Wo / BETA).reshape(T, H).reshape(T, KC, P).transpose(0, 2, 1)
    )                                                              # [T,P,KC]
    b0v_e = np.ascontiguousarray((BETA * b0).reshape(T, MC, P).transpose(0, 2, 1))
    bhv_e = np.ascontiguousarray(
        (BETA * bh).reshape(T, N_HID, MC, P).transpose(0, 3, 1, 2)
    )                                                              # [T,P,7,MC]
    bov_e = bo.reshape(T, 1)

    in_maps = []
    for c in range(N_CORES):
        sel = np.where(gidx[c] >= 0, gidx[c], 0)
        xg = x[sel]                                                # [NP, 67]
        ex = slot_expert[c]
        in_maps.append({
            "xT": np.ascontiguousarray(xg.T).astype(np_wdt),
            "w0t": w0t_e[ex].astype(np_wdt),
            "wht": wht_e[ex].astype(np_wdt),
            "wot": wot_e[ex].astype(np_wdt),
            "b0v": b0v_e[ex],
            "bhv": bhv_e[ex],
            "bov": bov_e[ex],
        })

    key = (shape, MM_MODE)
    if key not in _nc_cache:
        _nc_cache[key] = _build_nc(shape, MM_MODE)
    nc = _nc_cache[key]

    res = run_bass_kernel_spmd(nc, in_maps, core_ids=list(range(N_CORES)))
    global _last_results
    _last_results = res

    out = np.zeros((N, OUT), dtype=np.float32)
    for c in range(N_CORES):
        oc = res.results[c]["out"].reshape(-1)
        m = gidx[c] >= 0
        out[gidx[c][m], 0] = oc[m]
    return out
